# revision 40
# baseline (speedup 1.0000x reference)
"""BEVFusion LSS camera->BEV pooling on 8 Trainium2 NeuronCores.

Strategy (voxel-sorted streaming, paired leaves, hybrid one-hot):
- Host computes per-point voxel ids + kept mask (jax on CPU, mirroring the
  reference op-for-op; numpy fallback), sorts kept points by voxel, and
  pairs same-voxel points into level-1 slots (A/B leaf streams). Slots are
  packed into 128-slot chunks per 128-voxel window (gw), padding each
  window's chunk count to a multiple of L=2 so the device can run
  fixed-length PSUM accumulation chains with an input-independent
  instruction stream (one SPMD program on all 8 cores).
- Leaf features are cast to bf16 and laid out partition-major
  ([128, chunks*80]) so the device input is two pure sequential HWDGE
  streams at line rate (no dma_gather: a gather version was SWDGE-bound).
- Pooling per chunk: two matmuls (A leaf, B leaf) sharing one one-hot
  (slot -> voxel lane) as the stationary operand, accumulating
  [128vox, 80ch] in PSUM over L chunks (the pair-sum is absorbed into the
  PSUM accumulation; B leaves of singleton slots are zero rows).
  One-hot sourcing is hybrid to balance engine load: a fraction of
  32-chunk batches comes precomputed from the host as fp8 (DMA'd on the
  otherwise-idle SWDGE queue; fp8 x bf16 matmul is exact for 0/1
  weights), the rest is generated on DVE via is_equal(slot, iota).
- ACT copies 8 accumulated windows per instruction (strided PSUM read)
  into a bf16 staging ring; out-DMAs alternate between the two HWDGE
  rings.
- Host adds the per-group [128,80] blocks into the final [1,80,360,360]
  grid (pure unshard/assembly: each block -> its window's voxel range).
"""
import numpy as np
import ml_dtypes

# ---- problem geometry (hardcoded from the nn.Module config) ----
IMG_H, IMG_W = 256, 704
FH, FW = 32, 88
DBOUND = (1.0, 60.0, 0.5)
XB = (-54.0, 54.0, 0.3)
YB = (-54.0, 54.0, 0.3)
ZB = (-10.0, 10.0, 20.0)
NXX, NXY, NZ = 360, 360, 1
NVOX = NZ * NXX * NXY
NGW = (NVOX + 127) // 128
C = 80
N_CORES = 8
CHUNK = 128
L = 2          # chunks per PSUM accumulation chain (group)
TILE_G = 16    # groups per feature DMA tile (32 chunks, 0.66 MB bf16)
STAGE_G = 32   # groups per output staging buffer
BF_G = 16      # groups per one-hot batch (32 chunks, [128, 4096])
PS_G = 8       # groups per PSUM tile (4 banks, 256-col window spacing)
HB_P = 4       # host one-hot pattern period (in BF_G batches)
HB_K = 1       # trailing batches per period served by host fp8 one-hots

BF16 = ml_dtypes.bfloat16
FP8 = ml_dtypes.float8_e4m3

_last_results = None     # test.py introspection


def _compute_coords(lidar2camera, camera_intrinsics):
    try:
        return _compute_coords_jax(lidar2camera, camera_intrinsics)
    except Exception:
        return _compute_coords_np(lidar2camera, camera_intrinsics)


def _compute_coords_jax(lidar2camera, camera_intrinsics):
    import jax
    import jax.numpy as jnp

    with jax.default_device(jax.devices("cpu")[0]):
        l2c = jnp.asarray(np.asarray(lidar2camera, np.float32))
        K = jnp.asarray(np.asarray(camera_intrinsics, np.float32))
        cam2lidar = jnp.linalg.inv(l2c)
        rots = cam2lidar[..., :3, :3]
        trans = cam2lidar[..., :3, 3]
        intrins = K[..., :3, :3]
        ds = jnp.arange(*DBOUND, dtype=jnp.float32)
        D = ds.shape[0]
        xs = jnp.linspace(0.0, IMG_W - 1.0, FW, dtype=jnp.float32)
        ys = jnp.linspace(0.0, IMG_H - 1.0, FH, dtype=jnp.float32)
        ds_b = jnp.broadcast_to(ds[:, None, None], (D, FH, FW))
        xs_b = jnp.broadcast_to(xs[None, None, :], (D, FH, FW))
        ys_b = jnp.broadcast_to(ys[None, :, None], (D, FH, FW))
        frustum = jnp.stack((xs_b, ys_b, ds_b), axis=-1)
        pts = jnp.concatenate(
            [frustum[..., :2] * frustum[..., 2:3], frustum[..., 2:3]], axis=-1
        )
        combine = rots @ jnp.linalg.inv(intrins)
        geom = jnp.einsum("bnij,dhwj->bndhwi", combine, pts) + trans[
            :, :, None, None, None, :
        ]
        DX = jnp.array([XB[2], YB[2], ZB[2]], jnp.float32)
        BX = jnp.array(
            [XB[0] + XB[2] / 2.0, YB[0] + YB[2] / 2.0, ZB[0] + ZB[2] / 2.0],
            jnp.float32,
        )
        B, N = l2c.shape[0], l2c.shape[1]
        Nprime = B * N * D * FH * FW
        coords = ((geom.reshape(Nprime, 3) - (BX - DX / 2.0)) / DX).astype(jnp.int32)
        kept = (
            (coords[:, 0] >= 0) & (coords[:, 0] < NXX)
            & (coords[:, 1] >= 0) & (coords[:, 1] < NXY)
            & (coords[:, 2] >= 0) & (coords[:, 2] < NZ)
        )
        flat = (coords[:, 2] * NXX + coords[:, 0]) * NXY + coords[:, 1]
        return np.asarray(flat).astype(np.int64), np.asarray(kept)


def _compute_coords_np(lidar2camera, camera_intrinsics):
    l2c = np.asarray(lidar2camera, dtype=np.float32)
    K = np.asarray(camera_intrinsics, dtype=np.float32)
    cam2lidar = np.linalg.inv(l2c)
    rots = cam2lidar[..., :3, :3]
    trans = cam2lidar[..., :3, 3]
    intrins = K[..., :3, :3]
    ds = np.arange(*DBOUND, dtype=np.float32)
    D = ds.shape[0]
    xs = np.linspace(0.0, IMG_W - 1.0, FW, dtype=np.float32)
    ys = np.linspace(0.0, IMG_H - 1.0, FH, dtype=np.float32)
    ds_b = np.broadcast_to(ds[:, None, None], (D, FH, FW))
    xs_b = np.broadcast_to(xs[None, None, :], (D, FH, FW))
    ys_b = np.broadcast_to(ys[None, :, None], (D, FH, FW))
    frustum = np.stack((xs_b, ys_b, ds_b), axis=-1)
    pts = np.concatenate(
        [frustum[..., :2] * frustum[..., 2:3], frustum[..., 2:3]], axis=-1
    ).astype(np.float32)
    combine = (rots @ np.linalg.inv(intrins)).astype(np.float32)
    geom = np.einsum("bnij,dhwj->bndhwi", combine, pts, dtype=np.float32) + trans[
        :, :, None, None, None, :
    ]
    DX = np.array([XB[2], YB[2], ZB[2]], np.float32)
    BX = np.array(
        [XB[0] + XB[2] / 2.0, YB[0] + YB[2] / 2.0, ZB[0] + ZB[2] / 2.0], np.float32
    )
    B, N = l2c.shape[0], l2c.shape[1]
    Nprime = B * N * D * FH * FW
    coords = ((geom.reshape(Nprime, 3) - (BX - DX / 2.0)) / DX).astype(np.int32)
    kept = (
        (coords[:, 0] >= 0) & (coords[:, 0] < NXX)
        & (coords[:, 1] >= 0) & (coords[:, 1] < NXY)
        & (coords[:, 2] >= 0) & (coords[:, 2] < NZ)
    )
    flat = (coords[:, 2].astype(np.int64) * NXX + coords[:, 0]) * NXY + coords[:, 1]
    return flat, kept


def _plan(vox, kept):
    """Voxel-sorted level-1 slot stream: same-voxel points paired (A, B).

    Each level-1 slot holds up to 2 points of one voxel; the device pools
    both leaves with two matmuls sharing one one-hot (PSUM accumulates),
    halving chunk count, one-hot work, and output blocks.

    Returns (stream_rowA, stream_rowB, stream_slot, group_window, Gmax):
    - stream_rowA/B [8*Gmax*L*128] int64: source rows (-1 = absent/pad)
    - stream_slot   [8*Gmax*L*128] uint8: voxel lane 0..127 (255 = pad)
    - group_window  [8*Gmax] int64: window id of each L-chunk group (-1 = pad)
    - Gmax: groups per core, multiple of STAGE_G
    """
    rows_all = np.nonzero(kept)[0]
    v_kept = vox[rows_all]
    order = np.argsort(v_kept, kind="stable")
    v_sorted = v_kept[order]
    rows_sorted = rows_all[order]

    uniq, ustart, ucnt = np.unique(v_sorted, return_index=True, return_counts=True)
    s_v = (ucnt + 1) // 2                         # level-1 slots per voxel
    sbase = np.concatenate([[0], np.cumsum(s_v)])
    S = int(sbase[-1])
    idx_v = np.repeat(np.arange(len(uniq)), s_v)  # voxel index per slot
    r = np.arange(S, dtype=np.int64) - sbase[idx_v]
    A_pos = ustart[idx_v] + 2 * r
    B_pos = A_pos + 1
    B_valid = (2 * r + 1) < ucnt[idx_v]
    A_row = rows_sorted[A_pos]
    B_row = np.where(B_valid, rows_sorted[np.minimum(B_pos, len(rows_sorted) - 1)], -1)
    slot_voxel = uniq[idx_v]
    slot_lane = (slot_voxel & 127).astype(np.uint8)
    slot_gw = slot_voxel >> 7                     # window per slot (sorted order)

    sizes = np.bincount(slot_gw, minlength=NGW)   # slots per window
    cpg = (sizes + CHUNK - 1) // CHUNK            # chunks per window
    ppg = (cpg + L - 1) // L * L                  # padded to L-multiple
    total_groups = int(ppg.sum()) // L
    Gmax = (total_groups + N_CORES * STAGE_G - 1) // (N_CORES * STAGE_G) * STAGE_G
    Gtot = N_CORES * Gmax

    cbase = np.concatenate([[0], np.cumsum(ppg)])  # chunk base per window
    wstart = np.concatenate([[0], np.cumsum(sizes)])
    ranks = np.arange(S, dtype=np.int64) - wstart[slot_gw]
    pos = cbase[slot_gw] * CHUNK + ranks

    stream_rowA = np.full(Gtot * L * CHUNK, -1, np.int64)
    stream_rowB = np.full(Gtot * L * CHUNK, -1, np.int64)
    stream_slot = np.full(Gtot * L * CHUNK, 255, np.uint8)
    stream_rowA[pos] = A_row
    stream_rowB[pos] = B_row
    stream_slot[pos] = slot_lane

    group_window = np.full(Gtot, -1, np.int64)
    group_window[: total_groups] = np.repeat(
        np.arange(NGW, dtype=np.int64), (ppg // L)
    )
    return stream_rowA, stream_rowB, stream_slot, group_window, Gmax


def _host_batches(Gmax):
    """Indices of one-hot batches (BF_G groups each) served by host fp8.

    Host batches sit at the END of each period so the kernel's first
    batches are DVE-generated (no DMA wait on the critical ramp).
    """
    nb = Gmax // BF_G
    return [b for b in range(nb) if b % HB_P >= HB_P - HB_K]


def _leaf_array(x2d_bf16, rows):
    CTC = len(rows) // CHUNK
    feats = np.zeros((len(rows), C), BF16)
    m = rows >= 0
    feats[m] = x2d_bf16[rows[m]]
    return np.ascontiguousarray(
        feats.reshape(CTC, CHUNK, C).transpose(1, 0, 2).reshape(CHUNK, CTC * C)
    )


def _build_and_run(x2d_bf16, stream_rowA, stream_rowB, stream_slot, Gmax):
    import concourse.bass as bass  # noqa: F401
    import concourse.bacc as bacc
    import concourse.mybir as mybir
    import concourse.tile as tile
    from concourse.bass_utils import run_bass_kernel_spmd

    CT = Gmax * L                       # chunks per core
    assert Gmax % STAGE_G == 0 and Gmax % TILE_G == 0 and Gmax % BF_G == 0
    hbs = _host_batches(Gmax)
    n_hb = len(hbs)
    HB_CHUNKS = BF_G * L                # chunks per one-hot batch (32)

    in_maps = []
    iota = np.tile(np.arange(128, dtype=np.float32).astype(BF16), (128, 1))
    for k in range(N_CORES):
        lo, hi = k * CT * CHUNK, (k + 1) * CT * CHUNK
        feats = _leaf_array(x2d_bf16, stream_rowA[lo:hi])
        featsB = _leaf_array(x2d_bf16, stream_rowB[lo:hi])
        slot_cols = stream_slot[lo:hi].reshape(CT, CHUNK).T    # [128, CT] uint8
        slots = np.ascontiguousarray(slot_cols.astype(np.float32).astype(BF16))
        # host-precomputed fp8 one-hots for the selected batches
        hoh = np.zeros((CHUNK, max(1, n_hb * HB_CHUNKS * 128)), FP8)
        lanes = np.arange(128, dtype=np.int32)[None, None, :]
        for i, b in enumerate(hbs):
            sc = slot_cols[:, b * HB_CHUNKS:(b + 1) * HB_CHUNKS].astype(np.int32)
            blk = (sc[:, :, None] == lanes).astype(np.float32).astype(FP8)
            hoh[:, i * HB_CHUNKS * 128:(i + 1) * HB_CHUNKS * 128] = blk.reshape(
                CHUNK, HB_CHUNKS * 128
            )
        in_maps.append({"xs": feats, "xsb": featsB, "slots": slots,
                        "iota": iota, "hoh": hoh})

    nc = bacc.Bacc("TRN2", target_bir_lowering=False, debug=False,
                   num_devices=N_CORES, num_swdge_queues=4)
    xs_d = nc.declare_dram_parameter("xs", [CHUNK, CT * C], mybir.dt.bfloat16, isOutput=False)
    xsb_d = nc.declare_dram_parameter("xsb", [CHUNK, CT * C], mybir.dt.bfloat16, isOutput=False)
    slots_d = nc.declare_dram_parameter("slots", [CHUNK, CT], mybir.dt.bfloat16, isOutput=False)
    iota_d = nc.declare_dram_parameter("iota", [CHUNK, 128], mybir.dt.bfloat16, isOutput=False)
    hoh_d = nc.declare_dram_parameter("hoh", [CHUNK, max(1, n_hb * HB_CHUNKS * 128)], mybir.dt.float8e4, isOutput=False)
    out_d = nc.declare_dram_parameter("out", [CHUNK, Gmax * C], mybir.dt.bfloat16, isOutput=True)

    from concourse.tile import add_dep_helper

    with tile.TileContext(nc) as tc:
        with (
            tc.tile_pool(name="io", bufs=1) as io_pool,
            tc.tile_pool(name="feat", bufs=6) as f_pool,
            tc.tile_pool(name="oh", bufs=3) as oh_pool,
            tc.tile_pool(name="hoh", bufs=3) as hoh_pool,
            tc.tile_pool(name="stage", bufs=3) as st_pool,
            tc.tile_pool(name="psum", bufs=2, space="PSUM") as ps_pool,
        ):
            slot_t = io_pool.tile([CHUNK, CT], mybir.dt.bfloat16, tag="slots")
            i_slots = nc.sync.dma_start(out=slot_t[:], in_=slots_d[:])
            iota_t = io_pool.tile([CHUNK, 128], mybir.dt.bfloat16, tag="iota")
            i_iota = nc.sync.dma_start(out=iota_t[:], in_=iota_d[:])

            stage_t = None
            hb_seen = 0
            TW = TILE_G * L * C                # elems/partition per feat tile
            for t in range(Gmax // TILE_G):
                featA_t = f_pool.tile([CHUNK, TW], mybir.dt.bfloat16, tag="fta")
                featB_t = f_pool.tile([CHUNK, TW], mybir.dt.bfloat16, tag="ftb")
                f0 = t * TW
                i_a = nc.sync.dma_start(out=featA_t[:], in_=xs_d[:, f0:f0 + TW])
                nc.scalar.dma_start(out=featB_t[:], in_=xsb_d[:, f0:f0 + TW])
                if t == 0:
                    # keep the tiny slot/iota loads ahead of the bulk feature
                    # stream on the SP ring so the first one-hot fires early
                    add_dep_helper(i_a.ins, i_slots.ins, sync=False,
                                   reason="slots before A0")
                    add_dep_helper(i_a.ins, i_iota.ins, sync=False,
                                   reason="iota before A0")
                for bb in range(TILE_G // BF_G):
                    b = t * (TILE_G // BF_G) + bb
                    g0 = b * BF_G
                    host = b in hbs
                    if host:
                        oh = hoh_pool.tile(
                            [CHUNK, HB_CHUNKS * 128], mybir.dt.float8e4, tag="hoh"
                        )
                        o0 = hb_seen * HB_CHUNKS * 128
                        nc.gpsimd.dma_start(
                            out=oh[:], in_=hoh_d[:, o0:o0 + HB_CHUNKS * 128]
                        )
                        hb_seen += 1
                    else:
                        oh = oh_pool.tile(
                            [CHUNK, HB_CHUNKS * 128], mybir.dt.bfloat16, tag="oh"
                        )
                        nc.vector.tensor_tensor(
                            out=oh[:].rearrange("p (f s) -> p f s", s=128),
                            in0=slot_t[:, g0 * L:(g0 + BF_G) * L].to_broadcast(
                                [CHUNK, BF_G * L, 128]
                            ),
                            in1=iota_t[:].rearrange("p (f s) -> p f s", f=1)
                            .to_broadcast([CHUNK, BF_G * L, 128]),
                            op=mybir.AluOpType.is_equal,
                        )
                    for q in range(BF_G // PS_G):
                        ps = ps_pool.tile([CHUNK, PS_G * 256], mybir.dt.float32, tag="ps")
                        for gg in range(PS_G):
                            for jj in range(L):
                                cb = (q * PS_G + gg) * L + jj          # chunk in batch
                                jt = (bb * BF_G + q * PS_G + gg) * L + jj  # in tile
                                for leaf, ft in ((0, featA_t), (1, featB_t)):
                                    nc.tensor.matmul(
                                        out=ps[:, gg * 256:gg * 256 + C],
                                        lhsT=oh[:, cb * 128:(cb + 1) * 128],
                                        rhs=ft[:, jt * C:(jt + 1) * C],
                                        start=(jj == 0 and leaf == 0),
                                        stop=(jj == L - 1 and leaf == 1),
                                    )
                        gq = g0 + q * PS_G          # first group of this psum tile
                        r = gq % STAGE_G
                        if r == 0:
                            stage_t = st_pool.tile(
                                [CHUNK, STAGE_G * C], mybir.dt.bfloat16, tag="st"
                            )
                        nc.scalar.copy(
                            out=stage_t[:].rearrange("p (w x) -> p w x", x=C)[
                                :, r:r + PS_G
                            ],
                            in_=ps[:].rearrange("p (w x) -> p w x", x=256)[:, :, 0:C],
                        )
                        if r == STAGE_G - PS_G:
                            eng_o = nc.scalar if (gq // STAGE_G) % 2 == 0 else nc.sync
                            eng_o.dma_start(
                                out=out_d[:, (gq + PS_G - STAGE_G) * C:(gq + PS_G) * C],
                                in_=stage_t[:],
                            )

    nc.compile()
    res = run_bass_kernel_spmd(nc, in_maps, core_ids=list(range(N_CORES)))
    global _last_results
    _last_results = res
    return res


def kernel(x, lidar2camera, camera_intrinsics):
    x = np.asarray(x)
    B, N, D, H, W, C_ = x.shape
    assert (B, N, H, W, C_) == (1, 6, FH, FW, C), x.shape
    vox, kept = _compute_coords(lidar2camera, camera_intrinsics)
    stream_rowA, stream_rowB, stream_slot, group_window, Gmax = _plan(vox, kept)
    x2d_bf16 = np.ascontiguousarray(x.reshape(-1, C)).astype(BF16)
    res = _build_and_run(x2d_bf16, stream_rowA, stream_rowB, stream_slot, Gmax)

    grid = np.zeros((C, NGW * 128), np.float32)
    for k in range(N_CORES):
        out_k = np.asarray(res.results[k]["out"]).reshape(CHUNK, Gmax, C)
        gws = group_window[k * Gmax:(k + 1) * Gmax]
        for i in np.nonzero(gws >= 0)[0]:
            base = int(gws[i]) * 128
            grid[:, base:base + 128] += out_k[:, i, :].astype(np.float32).T
    return grid[:, :NVOX].reshape(1, C * NZ, NXX, NXY)


# revision 41
# speedup vs baseline: 1.0454x; 1.0454x over previous
"""BEVFusion LSS camera->BEV pooling on 8 Trainium2 NeuronCores.

Strategy (voxel-sorted streaming, paired leaves, hybrid one-hot):
- Host computes per-point voxel ids + kept mask (jax on CPU, mirroring the
  reference op-for-op; numpy fallback), sorts kept points by voxel, and
  pairs same-voxel points into level-1 slots (A/B leaf streams). Slots are
  packed into 128-slot chunks per 128-voxel window (gw), padding each
  window's chunk count to a multiple of L=2 so the device can run
  fixed-length PSUM accumulation chains with an input-independent
  instruction stream (one SPMD program on all 8 cores).
- Leaf features are cast to bf16 and laid out partition-major
  ([128, chunks*80]) so the device input is two pure sequential HWDGE
  streams at line rate (no dma_gather: a gather version was SWDGE-bound).
- Pooling per chunk: two matmuls (A leaf, B leaf) sharing one one-hot
  (slot -> voxel lane) as the stationary operand, accumulating
  [128vox, 80ch] in PSUM over L chunks (the pair-sum is absorbed into the
  PSUM accumulation; B leaves of singleton slots are zero rows).
  One-hot sourcing is hybrid to balance engine load: a fraction of
  32-chunk batches comes precomputed from the host as fp8 (DMA'd on the
  otherwise-idle SWDGE queue; fp8 x bf16 matmul is exact for 0/1
  weights), the rest is generated on DVE via is_equal(slot, iota).
- ACT copies 8 accumulated windows per instruction (strided PSUM read)
  into a bf16 staging ring; out-DMAs alternate between the two HWDGE
  rings.
- Host adds the per-group [128,80] blocks into the final [1,80,360,360]
  grid (pure unshard/assembly: each block -> its window's voxel range).
"""
import numpy as np
import ml_dtypes

# ---- problem geometry (hardcoded from the nn.Module config) ----
IMG_H, IMG_W = 256, 704
FH, FW = 32, 88
DBOUND = (1.0, 60.0, 0.5)
XB = (-54.0, 54.0, 0.3)
YB = (-54.0, 54.0, 0.3)
ZB = (-10.0, 10.0, 20.0)
NXX, NXY, NZ = 360, 360, 1
NVOX = NZ * NXX * NXY
NGW = (NVOX + 127) // 128
C = 80
N_CORES = 8
CHUNK = 128
L = 2          # chunks per PSUM accumulation chain (group)
TILE_G = 16    # groups per feature DMA tile (32 chunks, 0.66 MB bf16)
STAGE_G = 32   # groups per output staging buffer
BF_G = 16      # groups per one-hot batch (32 chunks, [128, 4096])
PS_G = 8       # groups per PSUM tile (4 banks, 256-col window spacing)
HB_P = 4       # host one-hot pattern period (in BF_G batches)
HB_K = 1       # trailing batches per period served by host fp8 one-hots

BF16 = ml_dtypes.bfloat16
FP8 = ml_dtypes.float8_e4m3

_last_results = None     # test.py introspection


def _compute_coords(lidar2camera, camera_intrinsics):
    try:
        return _compute_coords_jax(lidar2camera, camera_intrinsics)
    except Exception:
        return _compute_coords_np(lidar2camera, camera_intrinsics)


def _compute_coords_jax(lidar2camera, camera_intrinsics):
    import jax
    import jax.numpy as jnp

    with jax.default_device(jax.devices("cpu")[0]):
        l2c = jnp.asarray(np.asarray(lidar2camera, np.float32))
        K = jnp.asarray(np.asarray(camera_intrinsics, np.float32))
        cam2lidar = jnp.linalg.inv(l2c)
        rots = cam2lidar[..., :3, :3]
        trans = cam2lidar[..., :3, 3]
        intrins = K[..., :3, :3]
        ds = jnp.arange(*DBOUND, dtype=jnp.float32)
        D = ds.shape[0]
        xs = jnp.linspace(0.0, IMG_W - 1.0, FW, dtype=jnp.float32)
        ys = jnp.linspace(0.0, IMG_H - 1.0, FH, dtype=jnp.float32)
        ds_b = jnp.broadcast_to(ds[:, None, None], (D, FH, FW))
        xs_b = jnp.broadcast_to(xs[None, None, :], (D, FH, FW))
        ys_b = jnp.broadcast_to(ys[None, :, None], (D, FH, FW))
        frustum = jnp.stack((xs_b, ys_b, ds_b), axis=-1)
        pts = jnp.concatenate(
            [frustum[..., :2] * frustum[..., 2:3], frustum[..., 2:3]], axis=-1
        )
        combine = rots @ jnp.linalg.inv(intrins)
        geom = jnp.einsum("bnij,dhwj->bndhwi", combine, pts) + trans[
            :, :, None, None, None, :
        ]
        DX = jnp.array([XB[2], YB[2], ZB[2]], jnp.float32)
        BX = jnp.array(
            [XB[0] + XB[2] / 2.0, YB[0] + YB[2] / 2.0, ZB[0] + ZB[2] / 2.0],
            jnp.float32,
        )
        B, N = l2c.shape[0], l2c.shape[1]
        Nprime = B * N * D * FH * FW
        coords = ((geom.reshape(Nprime, 3) - (BX - DX / 2.0)) / DX).astype(jnp.int32)
        kept = (
            (coords[:, 0] >= 0) & (coords[:, 0] < NXX)
            & (coords[:, 1] >= 0) & (coords[:, 1] < NXY)
            & (coords[:, 2] >= 0) & (coords[:, 2] < NZ)
        )
        flat = (coords[:, 2] * NXX + coords[:, 0]) * NXY + coords[:, 1]
        return np.asarray(flat).astype(np.int64), np.asarray(kept)


def _compute_coords_np(lidar2camera, camera_intrinsics):
    l2c = np.asarray(lidar2camera, dtype=np.float32)
    K = np.asarray(camera_intrinsics, dtype=np.float32)
    cam2lidar = np.linalg.inv(l2c)
    rots = cam2lidar[..., :3, :3]
    trans = cam2lidar[..., :3, 3]
    intrins = K[..., :3, :3]
    ds = np.arange(*DBOUND, dtype=np.float32)
    D = ds.shape[0]
    xs = np.linspace(0.0, IMG_W - 1.0, FW, dtype=np.float32)
    ys = np.linspace(0.0, IMG_H - 1.0, FH, dtype=np.float32)
    ds_b = np.broadcast_to(ds[:, None, None], (D, FH, FW))
    xs_b = np.broadcast_to(xs[None, None, :], (D, FH, FW))
    ys_b = np.broadcast_to(ys[None, :, None], (D, FH, FW))
    frustum = np.stack((xs_b, ys_b, ds_b), axis=-1)
    pts = np.concatenate(
        [frustum[..., :2] * frustum[..., 2:3], frustum[..., 2:3]], axis=-1
    ).astype(np.float32)
    combine = (rots @ np.linalg.inv(intrins)).astype(np.float32)
    geom = np.einsum("bnij,dhwj->bndhwi", combine, pts, dtype=np.float32) + trans[
        :, :, None, None, None, :
    ]
    DX = np.array([XB[2], YB[2], ZB[2]], np.float32)
    BX = np.array(
        [XB[0] + XB[2] / 2.0, YB[0] + YB[2] / 2.0, ZB[0] + ZB[2] / 2.0], np.float32
    )
    B, N = l2c.shape[0], l2c.shape[1]
    Nprime = B * N * D * FH * FW
    coords = ((geom.reshape(Nprime, 3) - (BX - DX / 2.0)) / DX).astype(np.int32)
    kept = (
        (coords[:, 0] >= 0) & (coords[:, 0] < NXX)
        & (coords[:, 1] >= 0) & (coords[:, 1] < NXY)
        & (coords[:, 2] >= 0) & (coords[:, 2] < NZ)
    )
    flat = (coords[:, 2].astype(np.int64) * NXX + coords[:, 0]) * NXY + coords[:, 1]
    return flat, kept


def _plan(vox, kept):
    """Voxel-sorted level-1 slot stream: same-voxel points paired (A, B).

    Each level-1 slot holds up to 2 points of one voxel; the device pools
    both leaves with two matmuls sharing one one-hot (PSUM accumulates),
    halving chunk count, one-hot work, and output blocks.

    Returns (stream_rowA, stream_rowB, stream_slot, group_window, Gmax):
    - stream_rowA/B [8*Gmax*L*128] int64: source rows (-1 = absent/pad)
    - stream_slot   [8*Gmax*L*128] uint8: voxel lane 0..127 (255 = pad)
    - group_window  [8*Gmax] int64: window id of each L-chunk group (-1 = pad)
    - Gmax: groups per core, multiple of STAGE_G
    """
    rows_all = np.nonzero(kept)[0]
    v_kept = vox[rows_all]
    order = np.argsort(v_kept, kind="stable")
    v_sorted = v_kept[order]
    rows_sorted = rows_all[order]

    uniq, ustart, ucnt = np.unique(v_sorted, return_index=True, return_counts=True)
    s_v = (ucnt + 1) // 2                         # level-1 slots per voxel
    sbase = np.concatenate([[0], np.cumsum(s_v)])
    S = int(sbase[-1])
    idx_v = np.repeat(np.arange(len(uniq)), s_v)  # voxel index per slot
    r = np.arange(S, dtype=np.int64) - sbase[idx_v]
    A_pos = ustart[idx_v] + 2 * r
    B_pos = A_pos + 1
    B_valid = (2 * r + 1) < ucnt[idx_v]
    A_row = rows_sorted[A_pos]
    B_row = np.where(B_valid, rows_sorted[np.minimum(B_pos, len(rows_sorted) - 1)], -1)
    slot_voxel = uniq[idx_v]
    slot_lane = (slot_voxel & 127).astype(np.uint8)
    slot_gw = slot_voxel >> 7                     # window per slot (sorted order)

    sizes = np.bincount(slot_gw, minlength=NGW)   # slots per window
    cpg = (sizes + CHUNK - 1) // CHUNK            # chunks per window
    ppg = (cpg + L - 1) // L * L                  # padded to L-multiple
    total_groups = int(ppg.sum()) // L
    Gmax = (total_groups + N_CORES * STAGE_G - 1) // (N_CORES * STAGE_G) * STAGE_G
    Gtot = N_CORES * Gmax

    cbase = np.concatenate([[0], np.cumsum(ppg)])  # chunk base per window
    wstart = np.concatenate([[0], np.cumsum(sizes)])
    ranks = np.arange(S, dtype=np.int64) - wstart[slot_gw]
    pos = cbase[slot_gw] * CHUNK + ranks

    stream_rowA = np.full(Gtot * L * CHUNK, -1, np.int64)
    stream_rowB = np.full(Gtot * L * CHUNK, -1, np.int64)
    stream_slot = np.full(Gtot * L * CHUNK, 255, np.uint8)
    stream_rowA[pos] = A_row
    stream_rowB[pos] = B_row
    stream_slot[pos] = slot_lane

    group_window = np.full(Gtot, -1, np.int64)
    group_window[: total_groups] = np.repeat(
        np.arange(NGW, dtype=np.int64), (ppg // L)
    )
    return stream_rowA, stream_rowB, stream_slot, group_window, Gmax


def _host_batches(Gmax):
    """Indices of one-hot batches (BF_G groups each) served by host fp8.

    Host batches sit at the END of each period so the kernel's first
    batches are DVE-generated (no DMA wait on the critical ramp).
    """
    nb = Gmax // BF_G
    return [b for b in range(nb) if b % HB_P >= HB_P - HB_K]


def _leaf_array(x2d_bf16, rows):
    CTC = len(rows) // CHUNK
    feats = np.zeros((len(rows), C), BF16)
    m = rows >= 0
    feats[m] = x2d_bf16[rows[m]]
    return np.ascontiguousarray(
        feats.reshape(CTC, CHUNK, C).transpose(1, 0, 2).reshape(CHUNK, CTC * C)
    )


def _build_and_run(x2d_bf16, stream_rowA, stream_rowB, stream_slot, Gmax):
    import concourse.bass as bass  # noqa: F401
    import concourse.bacc as bacc
    import concourse.mybir as mybir
    import concourse.tile as tile
    from concourse.bass_utils import run_bass_kernel_spmd

    CT = Gmax * L                       # chunks per core
    assert Gmax % STAGE_G == 0 and Gmax % TILE_G == 0 and Gmax % BF_G == 0
    hbs = _host_batches(Gmax)
    n_hb = len(hbs)
    HB_CHUNKS = BF_G * L                # chunks per one-hot batch (32)

    in_maps = []
    iota = np.tile(np.arange(128, dtype=np.float32).astype(BF16), (128, 1))
    for k in range(N_CORES):
        lo, hi = k * CT * CHUNK, (k + 1) * CT * CHUNK
        feats = _leaf_array(x2d_bf16, stream_rowA[lo:hi])
        featsB = _leaf_array(x2d_bf16, stream_rowB[lo:hi])
        slot_cols = stream_slot[lo:hi].reshape(CT, CHUNK).T    # [128, CT] uint8
        slots = np.ascontiguousarray(slot_cols.astype(np.float32).astype(BF16))
        # host-precomputed fp8 one-hots for the selected batches
        hoh = np.zeros((CHUNK, max(1, n_hb * HB_CHUNKS * 128)), FP8)
        lanes = np.arange(128, dtype=np.int32)[None, None, :]
        for i, b in enumerate(hbs):
            sc = slot_cols[:, b * HB_CHUNKS:(b + 1) * HB_CHUNKS].astype(np.int32)
            blk = (sc[:, :, None] == lanes).astype(np.float32).astype(FP8)
            hoh[:, i * HB_CHUNKS * 128:(i + 1) * HB_CHUNKS * 128] = blk.reshape(
                CHUNK, HB_CHUNKS * 128
            )
        in_maps.append({"xs": feats, "xsb": featsB, "slots": slots,
                        "iota": iota, "hoh": hoh})

    nc = bacc.Bacc("TRN2", target_bir_lowering=False, debug=False,
                   num_devices=N_CORES, num_swdge_queues=4)
    xs_d = nc.declare_dram_parameter("xs", [CHUNK, CT * C], mybir.dt.bfloat16, isOutput=False)
    xsb_d = nc.declare_dram_parameter("xsb", [CHUNK, CT * C], mybir.dt.bfloat16, isOutput=False)
    slots_d = nc.declare_dram_parameter("slots", [CHUNK, CT], mybir.dt.bfloat16, isOutput=False)
    iota_d = nc.declare_dram_parameter("iota", [CHUNK, 128], mybir.dt.bfloat16, isOutput=False)
    hoh_d = nc.declare_dram_parameter("hoh", [CHUNK, max(1, n_hb * HB_CHUNKS * 128)], mybir.dt.float8e4, isOutput=False)
    out_d = nc.declare_dram_parameter("out", [CHUNK, Gmax * C], mybir.dt.bfloat16, isOutput=True)

    from concourse.tile import add_dep_helper

    with tile.TileContext(nc) as tc:
        with (
            tc.tile_pool(name="io", bufs=1) as io_pool,
            tc.tile_pool(name="feat", bufs=4) as f_pool,
            tc.tile_pool(name="oh", bufs=3) as oh_pool,
            tc.tile_pool(name="hoh", bufs=3) as hoh_pool,
            tc.tile_pool(name="stage", bufs=3) as st_pool,
            tc.tile_pool(name="psum", bufs=2, space="PSUM") as ps_pool,
        ):
            slot_t = io_pool.tile([CHUNK, CT], mybir.dt.bfloat16, tag="slots")
            i_slots = nc.sync.dma_start(out=slot_t[:], in_=slots_d[:])
            iota_t = io_pool.tile([CHUNK, 128], mybir.dt.bfloat16, tag="iota")
            i_iota = nc.sync.dma_start(out=iota_t[:], in_=iota_d[:])

            stage_t = None
            hb_seen = 0
            TW = TILE_G * L * C                # elems/partition per feat tile
            for t in range(Gmax // TILE_G):
                featA_t = f_pool.tile([CHUNK, TW], mybir.dt.bfloat16, tag="fta")
                featB_t = f_pool.tile([CHUNK, TW], mybir.dt.bfloat16, tag="ftb")
                f0 = t * TW
                i_a = nc.sync.dma_start(out=featA_t[:], in_=xs_d[:, f0:f0 + TW])
                nc.scalar.dma_start(out=featB_t[:], in_=xsb_d[:, f0:f0 + TW])
                if t == 0:
                    # keep the tiny slot/iota loads ahead of the bulk feature
                    # stream on the SP ring so the first one-hot fires early
                    add_dep_helper(i_a.ins, i_slots.ins, sync=False,
                                   reason="slots before A0")
                    add_dep_helper(i_a.ins, i_iota.ins, sync=False,
                                   reason="iota before A0")
                for bb in range(TILE_G // BF_G):
                    b = t * (TILE_G // BF_G) + bb
                    g0 = b * BF_G
                    host = b in hbs
                    if host:
                        oh = hoh_pool.tile(
                            [CHUNK, HB_CHUNKS * 128], mybir.dt.float8e4, tag="hoh"
                        )
                        o0 = hb_seen * HB_CHUNKS * 128
                        nc.gpsimd.dma_start(
                            out=oh[:], in_=hoh_d[:, o0:o0 + HB_CHUNKS * 128]
                        )
                        hb_seen += 1
                    else:
                        oh = oh_pool.tile(
                            [CHUNK, HB_CHUNKS * 128], mybir.dt.bfloat16, tag="oh"
                        )
                        nc.vector.tensor_tensor(
                            out=oh[:].rearrange("p (f s) -> p f s", s=128),
                            in0=slot_t[:, g0 * L:(g0 + BF_G) * L].to_broadcast(
                                [CHUNK, BF_G * L, 128]
                            ),
                            in1=iota_t[:].rearrange("p (f s) -> p f s", f=1)
                            .to_broadcast([CHUNK, BF_G * L, 128]),
                            op=mybir.AluOpType.is_equal,
                        )
                    for q in range(BF_G // PS_G):
                        ps = ps_pool.tile([CHUNK, PS_G * 256], mybir.dt.float32, tag="ps")
                        for gg in range(PS_G):
                            for jj in range(L):
                                cb = (q * PS_G + gg) * L + jj          # chunk in batch
                                jt = (bb * BF_G + q * PS_G + gg) * L + jj  # in tile
                                for leaf, ft in ((0, featA_t), (1, featB_t)):
                                    nc.tensor.matmul(
                                        out=ps[:, gg * 256:gg * 256 + C],
                                        lhsT=oh[:, cb * 128:(cb + 1) * 128],
                                        rhs=ft[:, jt * C:(jt + 1) * C],
                                        start=(jj == 0 and leaf == 0),
                                        stop=(jj == L - 1 and leaf == 1),
                                    )
                        gq = g0 + q * PS_G          # first group of this psum tile
                        r = gq % STAGE_G
                        if r == 0:
                            stage_t = st_pool.tile(
                                [CHUNK, STAGE_G * C], mybir.dt.bfloat16, tag="st"
                            )
                        nc.scalar.copy(
                            out=stage_t[:].rearrange("p (w x) -> p w x", x=C)[
                                :, r:r + PS_G
                            ],
                            in_=ps[:].rearrange("p (w x) -> p w x", x=256)[:, :, 0:C],
                        )
                        if r == STAGE_G - PS_G:
                            eng_o = nc.scalar if (gq // STAGE_G) % 2 == 0 else nc.sync
                            eng_o.dma_start(
                                out=out_d[:, (gq + PS_G - STAGE_G) * C:(gq + PS_G) * C],
                                in_=stage_t[:],
                            )

    nc.compile()
    res = run_bass_kernel_spmd(nc, in_maps, core_ids=list(range(N_CORES)))
    global _last_results
    _last_results = res
    return res


def kernel(x, lidar2camera, camera_intrinsics):
    x = np.asarray(x)
    B, N, D, H, W, C_ = x.shape
    assert (B, N, H, W, C_) == (1, 6, FH, FW, C), x.shape
    vox, kept = _compute_coords(lidar2camera, camera_intrinsics)
    stream_rowA, stream_rowB, stream_slot, group_window, Gmax = _plan(vox, kept)
    x2d_bf16 = np.ascontiguousarray(x.reshape(-1, C)).astype(BF16)
    res = _build_and_run(x2d_bf16, stream_rowA, stream_rowB, stream_slot, Gmax)

    grid = np.zeros((C, NGW * 128), np.float32)
    for k in range(N_CORES):
        out_k = np.asarray(res.results[k]["out"]).reshape(CHUNK, Gmax, C)
        gws = group_window[k * Gmax:(k + 1) * Gmax]
        for i in np.nonzero(gws >= 0)[0]:
            base = int(gws[i]) * 128
            grid[:, base:base + 128] += out_k[:, i, :].astype(np.float32).T
    return grid[:, :NVOX].reshape(1, C * NZ, NXX, NXY)


# revision 42
# speedup vs baseline: 1.0713x; 1.0248x over previous
"""BEVFusion LSS camera->BEV pooling on 8 Trainium2 NeuronCores.

Strategy (voxel-sorted streaming, paired leaves, hybrid one-hot):
- Host computes per-point voxel ids + kept mask (jax on CPU, mirroring the
  reference op-for-op; numpy fallback), sorts kept points by voxel, and
  pairs same-voxel points into level-1 slots (A/B leaf streams). Slots are
  packed into 128-slot chunks per 128-voxel window (gw), padding each
  window's chunk count to a multiple of L=2 so the device can run
  fixed-length PSUM accumulation chains with an input-independent
  instruction stream (one SPMD program on all 8 cores).
- Leaf features are cast to bf16 and laid out partition-major
  ([128, chunks*80]) so the device input is two pure sequential HWDGE
  streams at line rate (no dma_gather: a gather version was SWDGE-bound).
- Pooling per chunk: two matmuls (A leaf, B leaf) sharing one one-hot
  (slot -> voxel lane) as the stationary operand, accumulating
  [128vox, 80ch] in PSUM over L chunks (the pair-sum is absorbed into the
  PSUM accumulation; B leaves of singleton slots are zero rows).
  One-hot sourcing is hybrid to balance engine load: a fraction of
  32-chunk batches comes precomputed from the host as fp8 (DMA'd on the
  otherwise-idle SWDGE queue; fp8 x bf16 matmul is exact for 0/1
  weights), the rest is generated on DVE via is_equal(slot, iota).
- ACT copies 8 accumulated windows per instruction (strided PSUM read)
  into a bf16 staging ring; out-DMAs alternate between the two HWDGE
  rings.
- Host adds the per-group [128,80] blocks into the final [1,80,360,360]
  grid (pure unshard/assembly: each block -> its window's voxel range).
"""
import numpy as np
import ml_dtypes

# ---- problem geometry (hardcoded from the nn.Module config) ----
IMG_H, IMG_W = 256, 704
FH, FW = 32, 88
DBOUND = (1.0, 60.0, 0.5)
XB = (-54.0, 54.0, 0.3)
YB = (-54.0, 54.0, 0.3)
ZB = (-10.0, 10.0, 20.0)
NXX, NXY, NZ = 360, 360, 1
NVOX = NZ * NXX * NXY
NGW = (NVOX + 127) // 128
C = 80
N_CORES = 8
CHUNK = 128
L = 2          # chunks per PSUM accumulation chain (group)
TILE_G = 16    # groups per feature DMA tile (32 chunks, 0.66 MB bf16)
STAGE_G = 32   # groups per output staging buffer
BF_G = 16      # groups per one-hot batch (32 chunks, [128, 4096])
PS_G = 8       # groups per PSUM tile (4 banks, 256-col window spacing)
HB_P = 4       # host one-hot pattern period (in BF_G batches)
HB_K = 0       # trailing batches per period served by host fp8 one-hots

BF16 = ml_dtypes.bfloat16
FP8 = ml_dtypes.float8_e4m3

_last_results = None     # test.py introspection


def _compute_coords(lidar2camera, camera_intrinsics):
    try:
        return _compute_coords_jax(lidar2camera, camera_intrinsics)
    except Exception:
        return _compute_coords_np(lidar2camera, camera_intrinsics)


def _compute_coords_jax(lidar2camera, camera_intrinsics):
    import jax
    import jax.numpy as jnp

    with jax.default_device(jax.devices("cpu")[0]):
        l2c = jnp.asarray(np.asarray(lidar2camera, np.float32))
        K = jnp.asarray(np.asarray(camera_intrinsics, np.float32))
        cam2lidar = jnp.linalg.inv(l2c)
        rots = cam2lidar[..., :3, :3]
        trans = cam2lidar[..., :3, 3]
        intrins = K[..., :3, :3]
        ds = jnp.arange(*DBOUND, dtype=jnp.float32)
        D = ds.shape[0]
        xs = jnp.linspace(0.0, IMG_W - 1.0, FW, dtype=jnp.float32)
        ys = jnp.linspace(0.0, IMG_H - 1.0, FH, dtype=jnp.float32)
        ds_b = jnp.broadcast_to(ds[:, None, None], (D, FH, FW))
        xs_b = jnp.broadcast_to(xs[None, None, :], (D, FH, FW))
        ys_b = jnp.broadcast_to(ys[None, :, None], (D, FH, FW))
        frustum = jnp.stack((xs_b, ys_b, ds_b), axis=-1)
        pts = jnp.concatenate(
            [frustum[..., :2] * frustum[..., 2:3], frustum[..., 2:3]], axis=-1
        )
        combine = rots @ jnp.linalg.inv(intrins)
        geom = jnp.einsum("bnij,dhwj->bndhwi", combine, pts) + trans[
            :, :, None, None, None, :
        ]
        DX = jnp.array([XB[2], YB[2], ZB[2]], jnp.float32)
        BX = jnp.array(
            [XB[0] + XB[2] / 2.0, YB[0] + YB[2] / 2.0, ZB[0] + ZB[2] / 2.0],
            jnp.float32,
        )
        B, N = l2c.shape[0], l2c.shape[1]
        Nprime = B * N * D * FH * FW
        coords = ((geom.reshape(Nprime, 3) - (BX - DX / 2.0)) / DX).astype(jnp.int32)
        kept = (
            (coords[:, 0] >= 0) & (coords[:, 0] < NXX)
            & (coords[:, 1] >= 0) & (coords[:, 1] < NXY)
            & (coords[:, 2] >= 0) & (coords[:, 2] < NZ)
        )
        flat = (coords[:, 2] * NXX + coords[:, 0]) * NXY + coords[:, 1]
        return np.asarray(flat).astype(np.int64), np.asarray(kept)


def _compute_coords_np(lidar2camera, camera_intrinsics):
    l2c = np.asarray(lidar2camera, dtype=np.float32)
    K = np.asarray(camera_intrinsics, dtype=np.float32)
    cam2lidar = np.linalg.inv(l2c)
    rots = cam2lidar[..., :3, :3]
    trans = cam2lidar[..., :3, 3]
    intrins = K[..., :3, :3]
    ds = np.arange(*DBOUND, dtype=np.float32)
    D = ds.shape[0]
    xs = np.linspace(0.0, IMG_W - 1.0, FW, dtype=np.float32)
    ys = np.linspace(0.0, IMG_H - 1.0, FH, dtype=np.float32)
    ds_b = np.broadcast_to(ds[:, None, None], (D, FH, FW))
    xs_b = np.broadcast_to(xs[None, None, :], (D, FH, FW))
    ys_b = np.broadcast_to(ys[None, :, None], (D, FH, FW))
    frustum = np.stack((xs_b, ys_b, ds_b), axis=-1)
    pts = np.concatenate(
        [frustum[..., :2] * frustum[..., 2:3], frustum[..., 2:3]], axis=-1
    ).astype(np.float32)
    combine = (rots @ np.linalg.inv(intrins)).astype(np.float32)
    geom = np.einsum("bnij,dhwj->bndhwi", combine, pts, dtype=np.float32) + trans[
        :, :, None, None, None, :
    ]
    DX = np.array([XB[2], YB[2], ZB[2]], np.float32)
    BX = np.array(
        [XB[0] + XB[2] / 2.0, YB[0] + YB[2] / 2.0, ZB[0] + ZB[2] / 2.0], np.float32
    )
    B, N = l2c.shape[0], l2c.shape[1]
    Nprime = B * N * D * FH * FW
    coords = ((geom.reshape(Nprime, 3) - (BX - DX / 2.0)) / DX).astype(np.int32)
    kept = (
        (coords[:, 0] >= 0) & (coords[:, 0] < NXX)
        & (coords[:, 1] >= 0) & (coords[:, 1] < NXY)
        & (coords[:, 2] >= 0) & (coords[:, 2] < NZ)
    )
    flat = (coords[:, 2].astype(np.int64) * NXX + coords[:, 0]) * NXY + coords[:, 1]
    return flat, kept


def _plan(vox, kept):
    """Voxel-sorted level-1 slot stream: same-voxel points paired (A, B).

    Each level-1 slot holds up to 2 points of one voxel; the device pools
    both leaves with two matmuls sharing one one-hot (PSUM accumulates),
    halving chunk count, one-hot work, and output blocks.

    Returns (stream_rowA, stream_rowB, stream_slot, group_window, Gmax):
    - stream_rowA/B [8*Gmax*L*128] int64: source rows (-1 = absent/pad)
    - stream_slot   [8*Gmax*L*128] uint8: voxel lane 0..127 (255 = pad)
    - group_window  [8*Gmax] int64: window id of each L-chunk group (-1 = pad)
    - Gmax: groups per core, multiple of STAGE_G
    """
    rows_all = np.nonzero(kept)[0]
    v_kept = vox[rows_all]
    order = np.argsort(v_kept, kind="stable")
    v_sorted = v_kept[order]
    rows_sorted = rows_all[order]

    uniq, ustart, ucnt = np.unique(v_sorted, return_index=True, return_counts=True)
    s_v = (ucnt + 1) // 2                         # level-1 slots per voxel
    sbase = np.concatenate([[0], np.cumsum(s_v)])
    S = int(sbase[-1])
    idx_v = np.repeat(np.arange(len(uniq)), s_v)  # voxel index per slot
    r = np.arange(S, dtype=np.int64) - sbase[idx_v]
    A_pos = ustart[idx_v] + 2 * r
    B_pos = A_pos + 1
    B_valid = (2 * r + 1) < ucnt[idx_v]
    A_row = rows_sorted[A_pos]
    B_row = np.where(B_valid, rows_sorted[np.minimum(B_pos, len(rows_sorted) - 1)], -1)
    slot_voxel = uniq[idx_v]
    slot_lane = (slot_voxel & 127).astype(np.uint8)
    slot_gw = slot_voxel >> 7                     # window per slot (sorted order)

    sizes = np.bincount(slot_gw, minlength=NGW)   # slots per window
    cpg = (sizes + CHUNK - 1) // CHUNK            # chunks per window
    ppg = (cpg + L - 1) // L * L                  # padded to L-multiple
    total_groups = int(ppg.sum()) // L
    Gmax = (total_groups + N_CORES * STAGE_G - 1) // (N_CORES * STAGE_G) * STAGE_G
    Gtot = N_CORES * Gmax

    cbase = np.concatenate([[0], np.cumsum(ppg)])  # chunk base per window
    wstart = np.concatenate([[0], np.cumsum(sizes)])
    ranks = np.arange(S, dtype=np.int64) - wstart[slot_gw]
    pos = cbase[slot_gw] * CHUNK + ranks

    stream_rowA = np.full(Gtot * L * CHUNK, -1, np.int64)
    stream_rowB = np.full(Gtot * L * CHUNK, -1, np.int64)
    stream_slot = np.full(Gtot * L * CHUNK, 255, np.uint8)
    stream_rowA[pos] = A_row
    stream_rowB[pos] = B_row
    stream_slot[pos] = slot_lane

    group_window = np.full(Gtot, -1, np.int64)
    group_window[: total_groups] = np.repeat(
        np.arange(NGW, dtype=np.int64), (ppg // L)
    )
    return stream_rowA, stream_rowB, stream_slot, group_window, Gmax


def _host_batches(Gmax):
    """Indices of one-hot batches (BF_G groups each) served by host fp8.

    Host batches sit at the END of each period so the kernel's first
    batches are DVE-generated (no DMA wait on the critical ramp).
    """
    nb = Gmax // BF_G
    return [b for b in range(nb) if b % HB_P >= HB_P - HB_K]


def _leaf_array(x2d_bf16, rows):
    CTC = len(rows) // CHUNK
    feats = np.zeros((len(rows), C), BF16)
    m = rows >= 0
    feats[m] = x2d_bf16[rows[m]]
    return np.ascontiguousarray(
        feats.reshape(CTC, CHUNK, C).transpose(1, 0, 2).reshape(CHUNK, CTC * C)
    )


def _build_and_run(x2d_bf16, stream_rowA, stream_rowB, stream_slot, Gmax):
    import concourse.bass as bass  # noqa: F401
    import concourse.bacc as bacc
    import concourse.mybir as mybir
    import concourse.tile as tile
    from concourse.bass_utils import run_bass_kernel_spmd

    CT = Gmax * L                       # chunks per core
    assert Gmax % STAGE_G == 0 and Gmax % TILE_G == 0 and Gmax % BF_G == 0
    hbs = _host_batches(Gmax)
    n_hb = len(hbs)
    HB_CHUNKS = BF_G * L                # chunks per one-hot batch (32)

    in_maps = []
    iota = np.tile(np.arange(128, dtype=np.float32).astype(BF16), (128, 1))
    for k in range(N_CORES):
        lo, hi = k * CT * CHUNK, (k + 1) * CT * CHUNK
        feats = _leaf_array(x2d_bf16, stream_rowA[lo:hi])
        featsB = _leaf_array(x2d_bf16, stream_rowB[lo:hi])
        slot_cols = stream_slot[lo:hi].reshape(CT, CHUNK).T    # [128, CT] uint8
        slots = np.ascontiguousarray(slot_cols.astype(np.float32).astype(BF16))
        # host-precomputed fp8 one-hots for the selected batches
        hoh = np.zeros((CHUNK, max(1, n_hb * HB_CHUNKS * 128)), FP8)
        lanes = np.arange(128, dtype=np.int32)[None, None, :]
        for i, b in enumerate(hbs):
            sc = slot_cols[:, b * HB_CHUNKS:(b + 1) * HB_CHUNKS].astype(np.int32)
            blk = (sc[:, :, None] == lanes).astype(np.float32).astype(FP8)
            hoh[:, i * HB_CHUNKS * 128:(i + 1) * HB_CHUNKS * 128] = blk.reshape(
                CHUNK, HB_CHUNKS * 128
            )
        in_maps.append({"xs": feats, "xsb": featsB, "slots": slots,
                        "iota": iota, "hoh": hoh})

    nc = bacc.Bacc("TRN2", target_bir_lowering=False, debug=False,
                   num_devices=N_CORES, num_swdge_queues=4)
    xs_d = nc.declare_dram_parameter("xs", [CHUNK, CT * C], mybir.dt.bfloat16, isOutput=False)
    xsb_d = nc.declare_dram_parameter("xsb", [CHUNK, CT * C], mybir.dt.bfloat16, isOutput=False)
    slots_d = nc.declare_dram_parameter("slots", [CHUNK, CT], mybir.dt.bfloat16, isOutput=False)
    iota_d = nc.declare_dram_parameter("iota", [CHUNK, 128], mybir.dt.bfloat16, isOutput=False)
    hoh_d = nc.declare_dram_parameter("hoh", [CHUNK, max(1, n_hb * HB_CHUNKS * 128)], mybir.dt.float8e4, isOutput=False)
    out_d = nc.declare_dram_parameter("out", [CHUNK, Gmax * C], mybir.dt.bfloat16, isOutput=True)

    from concourse.tile import add_dep_helper

    with tile.TileContext(nc) as tc:
        with (
            tc.tile_pool(name="io", bufs=1) as io_pool,
            tc.tile_pool(name="feat", bufs=4) as f_pool,
            tc.tile_pool(name="oh", bufs=3) as oh_pool,
            tc.tile_pool(name="hoh", bufs=3) as hoh_pool,
            tc.tile_pool(name="stage", bufs=3) as st_pool,
            tc.tile_pool(name="psum", bufs=2, space="PSUM") as ps_pool,
        ):
            slot_t = io_pool.tile([CHUNK, CT], mybir.dt.bfloat16, tag="slots")
            i_slots = nc.sync.dma_start(out=slot_t[:], in_=slots_d[:])
            iota_t = io_pool.tile([CHUNK, 128], mybir.dt.bfloat16, tag="iota")
            i_iota = nc.sync.dma_start(out=iota_t[:], in_=iota_d[:])

            stage_t = None
            hb_seen = 0
            TW = TILE_G * L * C                # elems/partition per feat tile
            for t in range(Gmax // TILE_G):
                featA_t = f_pool.tile([CHUNK, TW], mybir.dt.bfloat16, tag="fta")
                featB_t = f_pool.tile([CHUNK, TW], mybir.dt.bfloat16, tag="ftb")
                f0 = t * TW
                i_a = nc.sync.dma_start(out=featA_t[:], in_=xs_d[:, f0:f0 + TW])
                nc.scalar.dma_start(out=featB_t[:], in_=xsb_d[:, f0:f0 + TW])
                if t == 0:
                    # keep the tiny slot/iota loads ahead of the bulk feature
                    # stream on the SP ring so the first one-hot fires early
                    add_dep_helper(i_a.ins, i_slots.ins, sync=False,
                                   reason="slots before A0")
                    add_dep_helper(i_a.ins, i_iota.ins, sync=False,
                                   reason="iota before A0")
                for bb in range(TILE_G // BF_G):
                    b = t * (TILE_G // BF_G) + bb
                    g0 = b * BF_G
                    host = b in hbs
                    if host:
                        oh = hoh_pool.tile(
                            [CHUNK, HB_CHUNKS * 128], mybir.dt.float8e4, tag="hoh"
                        )
                        o0 = hb_seen * HB_CHUNKS * 128
                        nc.gpsimd.dma_start(
                            out=oh[:], in_=hoh_d[:, o0:o0 + HB_CHUNKS * 128]
                        )
                        hb_seen += 1
                    else:
                        oh = oh_pool.tile(
                            [CHUNK, HB_CHUNKS * 128], mybir.dt.bfloat16, tag="oh"
                        )
                        nc.vector.tensor_tensor(
                            out=oh[:].rearrange("p (f s) -> p f s", s=128),
                            in0=slot_t[:, g0 * L:(g0 + BF_G) * L].to_broadcast(
                                [CHUNK, BF_G * L, 128]
                            ),
                            in1=iota_t[:].rearrange("p (f s) -> p f s", f=1)
                            .to_broadcast([CHUNK, BF_G * L, 128]),
                            op=mybir.AluOpType.is_equal,
                        )
                    for q in range(BF_G // PS_G):
                        ps = ps_pool.tile([CHUNK, PS_G * 256], mybir.dt.float32, tag="ps")
                        for gg in range(PS_G):
                            for jj in range(L):
                                cb = (q * PS_G + gg) * L + jj          # chunk in batch
                                jt = (bb * BF_G + q * PS_G + gg) * L + jj  # in tile
                                for leaf, ft in ((0, featA_t), (1, featB_t)):
                                    nc.tensor.matmul(
                                        out=ps[:, gg * 256:gg * 256 + C],
                                        lhsT=oh[:, cb * 128:(cb + 1) * 128],
                                        rhs=ft[:, jt * C:(jt + 1) * C],
                                        start=(jj == 0 and leaf == 0),
                                        stop=(jj == L - 1 and leaf == 1),
                                    )
                        gq = g0 + q * PS_G          # first group of this psum tile
                        r = gq % STAGE_G
                        if r == 0:
                            stage_t = st_pool.tile(
                                [CHUNK, STAGE_G * C], mybir.dt.bfloat16, tag="st"
                            )
                        nc.scalar.copy(
                            out=stage_t[:].rearrange("p (w x) -> p w x", x=C)[
                                :, r:r + PS_G
                            ],
                            in_=ps[:].rearrange("p (w x) -> p w x", x=256)[:, :, 0:C],
                        )
                        if r == STAGE_G - PS_G:
                            eng_o = nc.scalar if (gq // STAGE_G) % 2 == 0 else nc.sync
                            eng_o.dma_start(
                                out=out_d[:, (gq + PS_G - STAGE_G) * C:(gq + PS_G) * C],
                                in_=stage_t[:],
                            )

    nc.compile()
    res = run_bass_kernel_spmd(nc, in_maps, core_ids=list(range(N_CORES)))
    global _last_results
    _last_results = res
    return res


def kernel(x, lidar2camera, camera_intrinsics):
    x = np.asarray(x)
    B, N, D, H, W, C_ = x.shape
    assert (B, N, H, W, C_) == (1, 6, FH, FW, C), x.shape
    vox, kept = _compute_coords(lidar2camera, camera_intrinsics)
    stream_rowA, stream_rowB, stream_slot, group_window, Gmax = _plan(vox, kept)
    x2d_bf16 = np.ascontiguousarray(x.reshape(-1, C)).astype(BF16)
    res = _build_and_run(x2d_bf16, stream_rowA, stream_rowB, stream_slot, Gmax)

    grid = np.zeros((C, NGW * 128), np.float32)
    for k in range(N_CORES):
        out_k = np.asarray(res.results[k]["out"]).reshape(CHUNK, Gmax, C)
        gws = group_window[k * Gmax:(k + 1) * Gmax]
        for i in np.nonzero(gws >= 0)[0]:
            base = int(gws[i]) * 128
            grid[:, base:base + 128] += out_k[:, i, :].astype(np.float32).T
    return grid[:, :NVOX].reshape(1, C * NZ, NXX, NXY)


# revision 44
# speedup vs baseline: 1.2403x; 1.1577x over previous
"""BEVFusion LSS camera->BEV pooling on 8 Trainium2 NeuronCores.

Strategy (voxel-sorted streaming, paired leaves, hybrid one-hot):
- Host computes per-point voxel ids + kept mask (jax on CPU, mirroring the
  reference op-for-op; numpy fallback), sorts kept points by voxel, and
  pairs same-voxel points into level-1 slots (A/B leaf streams). Slots are
  packed into 128-slot chunks per 128-voxel window (gw), padding each
  window's chunk count to a multiple of L=2 so the device can run
  fixed-length PSUM accumulation chains with an input-independent
  instruction stream (one SPMD program on all 8 cores).
- Leaf features are cast to bf16 and laid out partition-major
  ([128, chunks*80]) so the device input is two pure sequential HWDGE
  streams at line rate (no dma_gather: a gather version was SWDGE-bound).
- Pooling per chunk: two matmuls (A leaf, B leaf) sharing one one-hot
  (slot -> voxel lane) as the stationary operand, accumulating
  [128vox, 80ch] in PSUM over L chunks (the pair-sum is absorbed into the
  PSUM accumulation; B leaves of singleton slots are zero rows).
  One-hot sourcing is hybrid to balance engine load: a fraction of
  32-chunk batches comes precomputed from the host as fp8 (DMA'd on the
  otherwise-idle SWDGE queue; fp8 x bf16 matmul is exact for 0/1
  weights), the rest is generated on DVE via is_equal(slot, iota).
- ACT copies 8 accumulated windows per instruction (strided PSUM read)
  into a bf16 staging ring; out-DMAs alternate between the two HWDGE
  rings.
- Host adds the per-group [128,80] blocks into the final [1,80,360,360]
  grid (pure unshard/assembly: each block -> its window's voxel range).
"""
import numpy as np
import ml_dtypes

# ---- problem geometry (hardcoded from the nn.Module config) ----
IMG_H, IMG_W = 256, 704
FH, FW = 32, 88
DBOUND = (1.0, 60.0, 0.5)
XB = (-54.0, 54.0, 0.3)
YB = (-54.0, 54.0, 0.3)
ZB = (-10.0, 10.0, 20.0)
NXX, NXY, NZ = 360, 360, 1
NVOX = NZ * NXX * NXY
NGW = (NVOX + 127) // 128
C = 80
N_CORES = 8
CHUNK = 128
TC = 32        # chunks per feature DMA tile (0.66 MB bf16)
OHC = 32       # chunks per one-hot DVE batch ([128, 4096])
PS_W = 8       # window chains per PSUM tile (4 banks, 256-col spacing)
STAGE_W = 16   # window blocks per output staging buffer

BF16 = ml_dtypes.bfloat16
FP8 = ml_dtypes.float8_e4m3

_last_results = None     # test.py introspection


def _compute_coords(lidar2camera, camera_intrinsics):
    try:
        return _compute_coords_jax(lidar2camera, camera_intrinsics)
    except Exception:
        return _compute_coords_np(lidar2camera, camera_intrinsics)


def _compute_coords_jax(lidar2camera, camera_intrinsics):
    import jax
    import jax.numpy as jnp

    with jax.default_device(jax.devices("cpu")[0]):
        l2c = jnp.asarray(np.asarray(lidar2camera, np.float32))
        K = jnp.asarray(np.asarray(camera_intrinsics, np.float32))
        cam2lidar = jnp.linalg.inv(l2c)
        rots = cam2lidar[..., :3, :3]
        trans = cam2lidar[..., :3, 3]
        intrins = K[..., :3, :3]
        ds = jnp.arange(*DBOUND, dtype=jnp.float32)
        D = ds.shape[0]
        xs = jnp.linspace(0.0, IMG_W - 1.0, FW, dtype=jnp.float32)
        ys = jnp.linspace(0.0, IMG_H - 1.0, FH, dtype=jnp.float32)
        ds_b = jnp.broadcast_to(ds[:, None, None], (D, FH, FW))
        xs_b = jnp.broadcast_to(xs[None, None, :], (D, FH, FW))
        ys_b = jnp.broadcast_to(ys[None, :, None], (D, FH, FW))
        frustum = jnp.stack((xs_b, ys_b, ds_b), axis=-1)
        pts = jnp.concatenate(
            [frustum[..., :2] * frustum[..., 2:3], frustum[..., 2:3]], axis=-1
        )
        combine = rots @ jnp.linalg.inv(intrins)
        geom = jnp.einsum("bnij,dhwj->bndhwi", combine, pts) + trans[
            :, :, None, None, None, :
        ]
        DX = jnp.array([XB[2], YB[2], ZB[2]], jnp.float32)
        BX = jnp.array(
            [XB[0] + XB[2] / 2.0, YB[0] + YB[2] / 2.0, ZB[0] + ZB[2] / 2.0],
            jnp.float32,
        )
        B, N = l2c.shape[0], l2c.shape[1]
        Nprime = B * N * D * FH * FW
        coords = ((geom.reshape(Nprime, 3) - (BX - DX / 2.0)) / DX).astype(jnp.int32)
        kept = (
            (coords[:, 0] >= 0) & (coords[:, 0] < NXX)
            & (coords[:, 1] >= 0) & (coords[:, 1] < NXY)
            & (coords[:, 2] >= 0) & (coords[:, 2] < NZ)
        )
        flat = (coords[:, 2] * NXX + coords[:, 0]) * NXY + coords[:, 1]
        return np.asarray(flat).astype(np.int64), np.asarray(kept)


def _compute_coords_np(lidar2camera, camera_intrinsics):
    l2c = np.asarray(lidar2camera, dtype=np.float32)
    K = np.asarray(camera_intrinsics, dtype=np.float32)
    cam2lidar = np.linalg.inv(l2c)
    rots = cam2lidar[..., :3, :3]
    trans = cam2lidar[..., :3, 3]
    intrins = K[..., :3, :3]
    ds = np.arange(*DBOUND, dtype=np.float32)
    D = ds.shape[0]
    xs = np.linspace(0.0, IMG_W - 1.0, FW, dtype=np.float32)
    ys = np.linspace(0.0, IMG_H - 1.0, FH, dtype=np.float32)
    ds_b = np.broadcast_to(ds[:, None, None], (D, FH, FW))
    xs_b = np.broadcast_to(xs[None, None, :], (D, FH, FW))
    ys_b = np.broadcast_to(ys[None, :, None], (D, FH, FW))
    frustum = np.stack((xs_b, ys_b, ds_b), axis=-1)
    pts = np.concatenate(
        [frustum[..., :2] * frustum[..., 2:3], frustum[..., 2:3]], axis=-1
    ).astype(np.float32)
    combine = (rots @ np.linalg.inv(intrins)).astype(np.float32)
    geom = np.einsum("bnij,dhwj->bndhwi", combine, pts, dtype=np.float32) + trans[
        :, :, None, None, None, :
    ]
    DX = np.array([XB[2], YB[2], ZB[2]], np.float32)
    BX = np.array(
        [XB[0] + XB[2] / 2.0, YB[0] + YB[2] / 2.0, ZB[0] + ZB[2] / 2.0], np.float32
    )
    B, N = l2c.shape[0], l2c.shape[1]
    Nprime = B * N * D * FH * FW
    coords = ((geom.reshape(Nprime, 3) - (BX - DX / 2.0)) / DX).astype(np.int32)
    kept = (
        (coords[:, 0] >= 0) & (coords[:, 0] < NXX)
        & (coords[:, 1] >= 0) & (coords[:, 1] < NXY)
        & (coords[:, 2] >= 0) & (coords[:, 2] < NZ)
    )
    flat = (coords[:, 2].astype(np.int64) * NXX + coords[:, 0]) * NXY + coords[:, 1]
    return flat, kept


def _plan(vox, kept):
    """Voxel-sorted paired slots, variable-length chains via a shared
    chain-length template.

    Same-voxel points are paired into level-1 slots (A/B leaves).  Whole
    windows (128-voxel blocks) are snake-dealt to the 8 cores by
    descending chunk count, so every core's rank-sorted window-size
    sequence is nearly identical; each rank is padded to the per-rank
    MAX over cores (the template).  All cores then share one instruction
    stream of WBAR variable-length PSUM chains (one output block per
    window, no L-padding, no window splits).

    Returns (rowsA, rowsB, slots8, win_id, template):
    - rowsA/rowsB [8, CTc*128] int64 source rows (-1 = absent/pad)
    - slots8 [8, CTc*128] uint8 voxel lane (255 = pad)
    - win_id [8, WBAR] int64 window of each chain (-1 = dummy)
    - template [WBAR] int chain length in chunks (>=1)
    """
    rows_all = np.nonzero(kept)[0]
    v_kept = vox[rows_all]
    order = np.argsort(v_kept, kind="stable")
    v_sorted = v_kept[order]
    rows_sorted = rows_all[order]

    uniq, ustart, ucnt = np.unique(v_sorted, return_index=True, return_counts=True)
    s_v = (ucnt + 1) // 2                         # level-1 slots per voxel
    sbase = np.concatenate([[0], np.cumsum(s_v)])
    S = int(sbase[-1])
    idx_v = np.repeat(np.arange(len(uniq)), s_v)
    r = np.arange(S, dtype=np.int64) - sbase[idx_v]
    A_pos = ustart[idx_v] + 2 * r
    B_valid = (2 * r + 1) < ucnt[idx_v]
    A_row = rows_sorted[A_pos]
    B_row = np.where(
        B_valid, rows_sorted[np.minimum(A_pos + 1, len(rows_sorted) - 1)], -1
    )
    slot_voxel = uniq[idx_v]
    slot_lane = (slot_voxel & 127).astype(np.uint8)
    slot_gw = slot_voxel >> 7

    sizes = np.bincount(slot_gw, minlength=NGW)   # slots per window
    wstart = np.concatenate([[0], np.cumsum(sizes)])
    nzw = np.nonzero(sizes)[0]
    cpg_w = (sizes[nzw] + CHUNK - 1) // CHUNK     # chunks per nonzero window
    wo = np.argsort(-cpg_w, kind="stable")        # big windows first
    wlists = [[] for _ in range(N_CORES)]
    for idx, oi in enumerate(wo):
        rr, cc = divmod(idx, N_CORES)
        k = cc if rr % 2 == 0 else N_CORES - 1 - cc
        wlists[k].append(oi)
    WBAR = (max(len(wl) for wl in wlists) + STAGE_W - 1) // STAGE_W * STAGE_W
    arr_c = np.zeros((N_CORES, WBAR), np.int64)
    for k in range(N_CORES):
        arr_c[k, :len(wlists[k])] = cpg_w[wlists[k]]
    template = np.maximum(arr_c.max(axis=0), 1)
    CTc = int(template.sum())
    cbase = np.concatenate([[0], np.cumsum(template)])

    rowsA = np.full((N_CORES, CTc * CHUNK), -1, np.int64)
    rowsB = np.full((N_CORES, CTc * CHUNK), -1, np.int64)
    slots8 = np.full((N_CORES, CTc * CHUNK), 255, np.uint8)
    win_id = np.full((N_CORES, WBAR), -1, np.int64)
    for k in range(N_CORES):
        for i, oi in enumerate(wlists[k]):
            w = int(nzw[oi])
            n = int(sizes[w])
            s0 = int(wstart[w])
            base = int(cbase[i]) * CHUNK
            rowsA[k, base:base + n] = A_row[s0:s0 + n]
            rowsB[k, base:base + n] = B_row[s0:s0 + n]
            slots8[k, base:base + n] = slot_lane[s0:s0 + n]
            win_id[k, i] = w
    return rowsA, rowsB, slots8, win_id, template


def _leaf_array(x2d_bf16, rows):
    CTC = len(rows) // CHUNK
    feats = np.zeros((len(rows), C), BF16)
    m = rows >= 0
    feats[m] = x2d_bf16[rows[m]]
    return np.ascontiguousarray(
        feats.reshape(CTC, CHUNK, C).transpose(1, 0, 2).reshape(CHUNK, CTC * C)
    )


def _build_and_run(x2d_bf16, rowsA, rowsB, slots8, template):
    import concourse.bass as bass  # noqa: F401
    import concourse.bacc as bacc
    import concourse.mybir as mybir
    import concourse.tile as tile
    from concourse.bass_utils import run_bass_kernel_spmd
    from concourse.tile import add_dep_helper

    WBAR = len(template)
    CTc = int(template.sum())
    assert WBAR % STAGE_W == 0

    in_maps = []
    iota = np.tile(np.arange(128, dtype=np.float32).astype(BF16), (128, 1))
    for k in range(N_CORES):
        feats = _leaf_array(x2d_bf16, rowsA[k])
        featsB = _leaf_array(x2d_bf16, rowsB[k])
        slots = np.ascontiguousarray(
            slots8[k].reshape(CTc, CHUNK).T.astype(np.float32).astype(BF16)
        )
        in_maps.append({"xs": feats, "xsb": featsB, "slots": slots, "iota": iota})

    nc = bacc.Bacc("TRN2", target_bir_lowering=False, debug=False,
                   num_devices=N_CORES)
    xs_d = nc.declare_dram_parameter("xs", [CHUNK, CTc * C], mybir.dt.bfloat16, isOutput=False)
    xsb_d = nc.declare_dram_parameter("xsb", [CHUNK, CTc * C], mybir.dt.bfloat16, isOutput=False)
    slots_d = nc.declare_dram_parameter("slots", [CHUNK, CTc], mybir.dt.bfloat16, isOutput=False)
    iota_d = nc.declare_dram_parameter("iota", [CHUNK, 128], mybir.dt.bfloat16, isOutput=False)
    out_d = nc.declare_dram_parameter("out", [CHUNK, WBAR * C], mybir.dt.bfloat16, isOutput=True)

    with tile.TileContext(nc) as tc:
        with (
            tc.tile_pool(name="io", bufs=1) as io_pool,
            tc.tile_pool(name="feat", bufs=4) as f_pool,
            tc.tile_pool(name="oh", bufs=3) as oh_pool,
            tc.tile_pool(name="stage", bufs=3) as st_pool,
            tc.tile_pool(name="psum", bufs=2, space="PSUM") as ps_pool,
        ):
            slot_t = io_pool.tile([CHUNK, CTc], mybir.dt.bfloat16, tag="slots")
            i_slots = nc.sync.dma_start(out=slot_t[:], in_=slots_d[:])
            iota_t = io_pool.tile([CHUNK, 128], mybir.dt.bfloat16, tag="iota")
            i_iota = nc.sync.dma_start(out=iota_t[:], in_=iota_d[:])

            pos = 0
            curA = curB = curoh = None
            stage_t = None
            ps = None
            nflush = 0
            for i in range(WBAR):
                s = i % PS_W
                if s == 0:
                    ps = ps_pool.tile([CHUNK, PS_W * 256], mybir.dt.float32, tag="ps")
                clen = int(template[i])
                for j in range(clen):
                    if pos % TC == 0:
                        t = pos // TC
                        w = min(TC, CTc - t * TC) * C
                        f0 = t * TC * C
                        curA = f_pool.tile([CHUNK, TC * C], mybir.dt.bfloat16, tag="fta")
                        curB = f_pool.tile([CHUNK, TC * C], mybir.dt.bfloat16, tag="ftb")
                        i_a = nc.sync.dma_start(out=curA[:, :w], in_=xs_d[:, f0:f0 + w])
                        nc.scalar.dma_start(out=curB[:, :w], in_=xsb_d[:, f0:f0 + w])
                        if t == 0:
                            add_dep_helper(i_a.ins, i_slots.ins, sync=False,
                                           reason="slots before A0")
                            add_dep_helper(i_a.ins, i_iota.ins, sync=False,
                                           reason="iota before A0")
                    if pos % OHC == 0:
                        b = pos // OHC
                        rem = min(OHC, CTc - b * OHC)
                        curoh = oh_pool.tile([CHUNK, OHC * 128], mybir.dt.bfloat16, tag="oh")
                        nc.vector.tensor_tensor(
                            out=curoh[:, :rem * 128].rearrange("p (f s) -> p f s", s=128),
                            in0=slot_t[:, b * OHC:b * OHC + rem].to_broadcast(
                                [CHUNK, rem, 128]
                            ),
                            in1=iota_t[:].rearrange("p (f s) -> p f s", f=1)
                            .to_broadcast([CHUNK, rem, 128]),
                            op=mybir.AluOpType.is_equal,
                        )
                    jt = pos % TC
                    cb = pos % OHC
                    for leaf, ft in ((0, curA), (1, curB)):
                        nc.tensor.matmul(
                            out=ps[:, s * 256:s * 256 + C],
                            lhsT=curoh[:, cb * 128:(cb + 1) * 128],
                            rhs=ft[:, jt * C:(jt + 1) * C],
                            start=(j == 0 and leaf == 0),
                            stop=(j == clen - 1 and leaf == 1),
                        )
                    pos += 1
                if s == PS_W - 1:
                    r = (i - (PS_W - 1)) % STAGE_W
                    if r == 0:
                        stage_t = st_pool.tile(
                            [CHUNK, STAGE_W * C], mybir.dt.bfloat16, tag="st"
                        )
                    nc.scalar.copy(
                        out=stage_t[:].rearrange("p (w x) -> p w x", x=C)[
                            :, r:r + PS_W
                        ],
                        in_=ps[:].rearrange("p (w x) -> p w x", x=256)[:, :, 0:C],
                    )
                    if (i + 1) % STAGE_W == 0:
                        eng_o = nc.scalar if nflush % 2 == 0 else nc.sync
                        nflush += 1
                        eng_o.dma_start(
                            out=out_d[:, (i + 1 - STAGE_W) * C:(i + 1) * C],
                            in_=stage_t[:],
                        )
            assert pos == CTc

    nc.compile()
    res = run_bass_kernel_spmd(nc, in_maps, core_ids=list(range(N_CORES)))
    global _last_results
    _last_results = res
    return res


def kernel(x, lidar2camera, camera_intrinsics):
    x = np.asarray(x)
    B, N, D, H, W, C_ = x.shape
    assert (B, N, H, W, C_) == (1, 6, FH, FW, C), x.shape
    vox, kept = _compute_coords(lidar2camera, camera_intrinsics)
    rowsA, rowsB, slots8, win_id, template = _plan(vox, kept)
    x2d_bf16 = np.ascontiguousarray(x.reshape(-1, C)).astype(BF16)
    res = _build_and_run(x2d_bf16, rowsA, rowsB, slots8, template)

    WBAR = len(template)
    grid = np.zeros((C, NGW * 128), np.float32)
    for k in range(N_CORES):
        out_k = np.asarray(res.results[k]["out"]).reshape(CHUNK, WBAR, C)
        for i in np.nonzero(win_id[k] >= 0)[0]:
            base = int(win_id[k][i]) * 128
            grid[:, base:base + 128] = out_k[:, i, :].astype(np.float32).T
    return grid[:, :NVOX].reshape(1, C * NZ, NXX, NXY)
